# revision 49
# baseline (speedup 1.0000x reference)
"""Trainium2 Bass kernel for the 3-layer GATv2 network (nn_GAT_35940286333219).

v4 (~935us vs 2111us v3 baseline): host-precomputed scatter masks in both
orientations streamed from DRAM (no on-device mask gen / PE transposes /
PSUM->SBUF mask copies); w = xl[src] + xr[dst] formed entirely on the PE as
two accumulating matmuls (mask-transpose gather + identity add); leaky-relu
via Prelu on the scalar engine; logits via one scalar_tensor_tensor
sign-multiply with free DVE accumulator (plus a per-head tensor_reduce for
the 4-head layer); |att| column scales folded into the tables with the
positive unscale applied past the ReLU inside the transposed residual add
(scalar_tensor_tensor from the transpose PSUM, no perm needed); alpha column
batched per slot into a [*, 257]-stride value buffer; per-slot gathers kept
at <=1024 descriptors (larger single gathers crash SWDGE); layer-1 table
slot-major with one AllGather, layers 2/3 chunk-major with a 12+4-slot
chunked AllGather (AG_CH) overlapping the xl/xr prep. Note: overlapping
next-layer gathers into the tail collective (src-chunk-sorted split gathers)
REGRESSED 25% - concurrent DMA-to-SBUF writes slow every DVE op ~30%.
"""
import os
import numpy as np
import ml_dtypes

import concourse.bacc as bacc
import concourse.bass as bass
import concourse.mybir as mybir
import concourse.tile as tile
from concourse.bass_utils import run_bass_kernel_spmd
from concourse.masks import make_identity

P = 128
N = 16384
NCORES = 8
NLOC = N // NCORES          # 2048
NBLK = NLOC // P            # 16 slots per core
F_IN = 128
DIM = 64
HID = 256
FP = mybir.dt.float32
BF = mybir.dt.bfloat16
I16 = mybir.dt.int16
AF = mybir.ActivationFunctionType
ALU = mybir.AluOpType
AX = mybir.AxisListType
BF_NP = ml_dtypes.bfloat16
NEG = 0.2
AG_CH = (0, 14, 16)         # AllGather chunk boundaries (slots)

LAST_RESULTS = None


def _prep_edges(edge_index):
    """Sort edges by dst, group per dst-block, per-core sort blocks by size
    (slot order), pad each slot to the max tile count across cores.

    Returns per-core idx arrays (int16, dma_gather wrap layout), host-built
    scatter masks in both orientations, NT per slot, table row map, and the
    local node order per core."""
    src = np.concatenate([edge_index[0], np.arange(N, dtype=np.int64)])
    dst = np.concatenate([edge_index[1], np.arange(N, dtype=np.int64)])
    order = np.argsort(dst, kind="stable")
    src_s, dst_s = src[order], dst[order]
    blk = dst_s // P                                  # global block 0..127
    bc = np.bincount(blk, minlength=NCORES * NBLK)
    starts = np.concatenate([[0], np.cumsum(bc)])

    # per-core slot order: blocks sorted by count desc
    slot_blocks = np.empty((NCORES, NBLK), np.int64)   # slot -> local block
    for c in range(NCORES):
        cnt = bc[c * NBLK:(c + 1) * NBLK]
        slot_blocks[c] = np.argsort(-cnt, kind="stable")
    # uniform tile count per slot = max over cores
    NT_slots = []
    for s in range(NBLK):
        ks = [bc[c * NBLK + slot_blocks[c, s]] for c in range(NCORES)]
        NT_slots.append(int(np.ceil(max(ks) / P)))
    EBLK = [nt * P for nt in NT_slots]

    # table row maps: node -> row in xl_full. Layer 1's table is slot-major
    # (one AllGather at startup); layers 2/3 are chunk-major (boundaries
    # AG_CH in slots) so their AllGathers run in chunks overlapping the
    # per-slot xl/xr prep.
    rownum1 = np.empty(N, np.int64)
    rownum23 = np.empty(N, np.int64)
    node_order = np.empty((NCORES, NLOC), np.int64)    # local idx -> node id
    for c in range(NCORES):
        for s in range(NBLK):
            b = slot_blocks[c, s]
            nodes = c * NLOC + b * P + np.arange(P)
            rownum1[nodes] = c * NLOC + s * P + np.arange(P)
            q = sum(1 for ch in AG_CH[1:-1] if s >= ch)
            pref = AG_CH[q] * P
            crows = (AG_CH[q + 1] - AG_CH[q]) * P
            rownum23[nodes] = (pref * NCORES + c * crows
                               + (s - AG_CH[q]) * P + np.arange(P))
            node_order[c, s * P:(s + 1) * P] = nodes

    tot_idx_cols = sum(e // 16 for e in EBLK)
    tot_nt = sum(NT_slots)
    idx1_all = np.zeros((NCORES, P, tot_idx_cols), np.int16)
    idx23_all = np.zeros((NCORES, P, tot_idx_cols), np.int16)
    m_host = np.zeros((NCORES, P, tot_nt * P), BF_NP)
    mT_host = np.zeros((NCORES, P, tot_nt * P), BF_NP)
    eye129 = np.zeros((P + 1, P), np.float32)
    eye129[np.arange(P), np.arange(P)] = 1.0
    for c in range(NCORES):
        ioff = noff = 0
        for s in range(NBLK):
            b = slot_blocks[c, s]
            g = c * NBLK + b
            lo, hi = starts[g], starts[g + 1]
            k = hi - lo
            e = EBLK[s]
            nt = NT_slots[s]
            for rn, ia in ((rownum1, idx1_all), (rownum23, idx23_all)):
                srcrow = np.zeros(e, np.int64)         # pads gather row 0
                srcrow[:k] = rn[src_s[lo:hi]]
                wrapped = srcrow.reshape(e // 16, 16).T
                ia[c, :, ioff:ioff + e // 16] = np.tile(wrapped, (8, 1))
            dcol = np.full(e, P, np.int64)             # pad -> zero mask row
            dcol[:k] = dst_s[lo:hi] - g * P
            oh = eye129[dcol]                          # [e, P]
            oh3 = oh.reshape(nt, P, P)
            m_host[c, :, noff * P:(noff + nt) * P] = (
                oh3.transpose(1, 0, 2).reshape(P, nt * P))
            mT_host[c, :, noff * P:(noff + nt) * P] = (
                oh3.transpose(2, 0, 1).reshape(P, nt * P))
            ioff += e // 16
            noff += nt
    return (idx1_all, idx23_all, m_host, mT_host, tuple(NT_slots), node_order)


def _prep_weights(ii):
    """Fold |att| into the Wl/Wr columns (positive scale); keep the sign in a
    broadcast row for the logit reduce, and the reciprocal scale (applied past
    the ReLU) for the transposed residual add."""
    out = {}
    for l in (1, 2, 3):
        Wl = np.asarray(ii[f"Wl{l}"], np.float32)
        Wr = np.asarray(ii[f"Wr{l}"], np.float32)
        att = np.asarray(ii[f"att{l}"], np.float32).reshape(-1)
        sc = np.maximum(np.abs(att), 1e-6)
        out[f"WL{l}"] = (Wl * sc[None, :]).astype(BF_NP)
        out[f"WR{l}"] = (Wr * sc[None, :]).astype(BF_NP)
        out[f"sgn{l}"] = np.tile(np.sign(att)[None, :].astype(BF_NP), (P, 1))
        out[f"dinv{l}"] = np.ascontiguousarray(
            (1.0 / sc).reshape(2, P).T.astype(np.float32))   # [P, 2]
    return out


def _build(NT_slots):
    nc = bacc.Bacc(None, num_swdge_queues=4)
    EBLK = [nt * P for nt in NT_slots]
    NTMAX = max(NT_slots)
    tot_idx_cols = sum(e // 16 for e in EBLK)
    tot_nt = sum(NT_slots)

    def par(name, shape, dtype=BF):
        return nc.declare_dram_parameter(name, list(shape), dtype, isOutput=False)

    xT = par("xT", [F_IN, NLOC])
    idx1 = par("idx1", [P, tot_idx_cols], I16)
    idx23 = par("idx23", [P, tot_idx_cols], I16)
    m_dram = par("m_dram", [P, tot_nt * P])
    mT_dram = par("mT_dram", [P, tot_nt * P])
    Win = par("Win", [F_IN, DIM]); b_in = par("b_in", [1, DIM])
    Wskip = par("Wskip", [DIM, HID]); bskip = par("bskip", [1, HID])
    WL1 = par("WL1", [DIM, HID]); WR1 = par("WR1", [DIM, HID])
    WL2 = par("WL2", [HID, HID]); WR2 = par("WR2", [HID, HID])
    WL3 = par("WL3", [HID, HID]); WR3 = par("WR3", [HID, HID])
    sgn = {l: par(f"sgn{l}", [P, HID]) for l in (1, 2, 3)}
    dinv = {l: par(f"dinv{l}", [P, 2], FP) for l in (1, 2, 3)}
    Wm1 = par("Wm1", [HID, DIM]); bm1 = par("bm1", [1, DIM])
    Wm2 = par("Wm2", [DIM, DIM]); bm2 = par("bm2", [1, DIM])
    Wm3 = par("Wm3", [DIM, 1]); bm3 = par("bm3", [1, 1])
    out = nc.declare_dram_parameter("out", [1, NLOC], FP, isOutput=True)

    xl_loc = {l: nc.dram_tensor(f"xl_loc{l}", [NLOC, HID], BF) for l in (1, 2, 3)}
    xl_full = {l: nc.dram_tensor(f"xl_full{l}", [N, HID], BF, addr_space="Shared")
               for l in (1, 2, 3)}

    with tile.TileContext(nc) as tc:
        with (
            tc.tile_pool(name="const", bufs=1) as cp,
            tc.tile_pool(name="big", bufs=1) as bigp,
            tc.tile_pool(name="wk", bufs=1) as wk,
            tc.tile_pool(name="ps_mm", bufs=2, space="PSUM") as ps_mm,
            tc.tile_pool(name="ps_w", bufs=3, space="PSUM") as ps_w,
            tc.tile_pool(name="ps_o", bufs=2, space="PSUM") as ps_o_pool,
            tc.tile_pool(name="ps_tp", bufs=1, space="PSUM") as ps_tp,
        ):
            def load_const(pname, ap, shape, dtype=BF):
                t = cp.tile(list(shape), dtype, name=pname + "_sb")
                nc.sync.dma_start(out=t[:], in_=ap[:])
                return t

            def load_const_2k(pname, ap, cols):
                t = cp.tile([P, 2 * cols], BF, name=pname + "_sb")
                nc.sync.dma_start(out=t[:, :cols], in_=ap[:P, :])
                nc.sync.dma_start(out=t[:, cols:], in_=ap[P:, :])
                return t

            ident_f = cp.tile([P, P], FP, name="ident_f")
            make_identity(nc, ident_f[:])
            ident_b = cp.tile([P, P], BF, name="ident_b")
            nc.vector.tensor_copy(out=ident_b[:], in_=ident_f[:])
            ones_row = cp.tile([1, 512], BF, name="ones_row")
            nc.vector.memset(ones_row[:], 1.0)
            ones_col = cp.tile([P, 1], BF, name="ones_col")
            nc.vector.memset(ones_col[:], 1.0)

            xT_sb = load_const("xT", xT, [F_IN, NLOC])
            idx_sb = {1: load_const("idx1", idx1, [P, tot_idx_cols], I16),
                      2: load_const("idx23", idx23, [P, tot_idx_cols], I16)}
            idx_sb[3] = idx_sb[2]
            Win_sb = load_const("Win", Win, [F_IN, DIM])
            b_in_sb = load_const("b_in", b_in, [1, DIM])
            Wskip_sb = load_const("Wskip", Wskip, [DIM, HID])
            bskip_sb = load_const("bskip", bskip, [1, HID])
            WL_sb = {1: load_const("WL1", WL1, [DIM, HID]),
                     2: load_const_2k("WL2", WL2, HID),
                     3: load_const_2k("WL3", WL3, HID)}
            WR_sb = {1: load_const("WR1", WR1, [DIM, HID]),
                     2: load_const_2k("WR2", WR2, HID),
                     3: load_const_2k("WR3", WR3, HID)}
            sgn_sb = {l: load_const(f"sgn{l}", sgn[l], [P, HID])
                      for l in (1, 2, 3)}
            dinv_sb = {l: load_const(f"dinv{l}", dinv[l], [P, 2], FP)
                       for l in (1, 2, 3)}
            Wm1_sb = load_const_2k("Wm1", Wm1, DIM)
            bm1_sb = load_const("bm1", bm1, [1, DIM])
            Wm2_sb = load_const("Wm2", Wm2, [DIM, DIM])
            bm2_sb = load_const("bm2", bm2, [1, DIM])
            Wm3_sb = load_const("Wm3", Wm3, [DIM, 1])
            bm3_sb = load_const("bm3", bm3, [1, 1])

            # transposed residual stream buffers (feature chunk k on partitions)
            sT = [bigp.tile([P, NLOC], BF, name=f"sT{k}") for k in range(2)]
            aT = [bigp.tile([P, NLOC], BF, name=f"aT{k}") for k in range(2)]
            bT = [bigp.tile([P, NLOC], BF, name=f"bT{k}") for k in range(2)]
            XRb = {0: bigp.tile([P, NBLK * HID], BF, name="XRb0"),
                   1: bigp.tile([P, NBLK * HID], BF, name="XRb1")}
            hT = bigp.tile([DIM, NLOC], BF, name="hT")
            m1T = bigp.tile([DIM, NLOC], BF, name="m1T")
            m2T = bigp.tile([DIM, NLOC], BF, name="m2T")
            y_sb = bigp.tile([1, NLOC], FP, name="y_sb")

            idx_off = [0]
            nt_off = [0]
            for s in range(NBLK):
                idx_off.append(idx_off[-1] + EBLK[s] // 16)
                nt_off.append(nt_off[-1] + NT_slots[s])

            def ag(l, s):
                if l == 1:                 # slot-major table, one collective
                    if s == NBLK - 1:
                        nc.gpsimd.collective_compute(
                            "AllGather", ALU.bypass,
                            replica_groups=[list(range(NCORES))],
                            ins=[xl_loc[1][:]], outs=[xl_full[1][:]])
                    return
                if s + 1 not in AG_CH:
                    return
                q = AG_CH.index(s + 1) - 1
                lo, hi = AG_CH[q] * P, AG_CH[q + 1] * P
                nc.gpsimd.collective_compute(
                    "AllGather", ALU.bypass,
                    replica_groups=[list(range(NCORES))],
                    ins=[xl_loc[l][lo:hi, :]],
                    outs=[xl_full[l][lo * NCORES:hi * NCORES, :]])

            def xlxr_slot(l, s, src0, src1):
                """xl/xr for layer l, slot s, from actT chunks src0/src1
                (or hT when l == 1); stores xl row-block, fills XRb."""
                nsl = slice(s * P, (s + 1) * P)
                pxl = ps_mm.tile([P, HID], FP, space="PSUM", name="pxl", tag="pmm")
                pxr = ps_mm.tile([P, HID], FP, space="PSUM", name="pxr", tag="pmm")
                if l == 1:
                    nc.tensor.matmul(out=pxl[:], lhsT=hT[:DIM, nsl], rhs=WL_sb[1][:],
                                     start=True, stop=True)
                    nc.tensor.matmul(out=pxr[:], lhsT=hT[:DIM, nsl], rhs=WR_sb[1][:],
                                     start=True, stop=True)
                else:
                    srcs = (src0, src1)
                    for k in range(2):
                        nc.tensor.matmul(out=pxl[:], lhsT=srcs[k][:, nsl],
                                         rhs=WL_sb[l][:, k * HID:(k + 1) * HID],
                                         start=(k == 0), stop=(k == 1))
                    for k in range(2):
                        nc.tensor.matmul(out=pxr[:], lhsT=srcs[k][:, nsl],
                                         rhs=WR_sb[l][:, k * HID:(k + 1) * HID],
                                         start=(k == 0), stop=(k == 1))
                xst = wk.tile([P, HID], BF, name="xst", tag="xst", bufs=4)
                nc.scalar.activation(out=xst[:], in_=pxl[:], func=AF.Copy)
                nc.sync.dma_start(out=xl_loc[l][nsl, :], in_=xst[:])
                nc.scalar.activation(out=XRb[l % 2][:, s * HID:(s + 1) * HID],
                                     in_=pxr[:], func=AF.Copy)

            # ---------------- phase A ----------------
            for j in range(NLOC // 512):
                sl = slice(j * 512, (j + 1) * 512)
                pmm = ps_mm.tile([P, 512], FP, space="PSUM", name="pmm", tag="pmm")
                nc.tensor.matmul(out=pmm[:DIM, :], lhsT=Win_sb[:], rhs=xT_sb[:, sl],
                                 start=True, stop=False)
                nc.tensor.matmul(out=pmm[:DIM, :], lhsT=b_in_sb[:], rhs=ones_row[:],
                                 start=False, stop=True)
                nc.scalar.activation(out=hT[:DIM, sl], in_=pmm[:DIM, :], func=AF.Relu)

            for s in range(NBLK):
                xlxr_slot(1, s, None, None)
                ag(1, s)

            # skip projection overlaps the layer-1 AllGather
            for k in range(2):
                ksl = slice(k * P, (k + 1) * P)
                for j in range(NLOC // 512):
                    sl = slice(j * 512, (j + 1) * 512)
                    psk = ps_mm.tile([P, 512], FP, space="PSUM", name="psk", tag="pmm")
                    nc.tensor.matmul(out=psk[:], lhsT=Wskip_sb[:, ksl],
                                     rhs=hT[:DIM, sl], start=True, stop=False)
                    nc.tensor.matmul(out=psk[:], lhsT=bskip_sb[:, ksl],
                                     rhs=ones_row[:], start=False, stop=True)
                    nc.scalar.activation(out=sT[k][:, sl], in_=psk[:], func=AF.Copy)

            # ---------------- edge stage ----------------
            def edge_layer(l, H, prev0, prev1, next0, next1):
                for s in range(NBLK):
                    NT = NT_slots[s]
                    m_sb = wk.tile([P, NTMAX * P], BF, name="m_sb",
                                   tag="m_sb", bufs=3)
                    nc.sync.dma_start(
                        out=m_sb[:, :NT * P],
                        in_=m_dram[:, nt_off[s] * P:nt_off[s + 1] * P])
                    mT_sb = wk.tile([P, NTMAX * P], BF, name="mT_sb",
                                    tag="mT_sb", bufs=3)
                    nc.sync.dma_start(
                        out=mT_sb[:, :NT * P],
                        in_=mT_dram[:, nt_off[s] * P:nt_off[s + 1] * P])
                    xl_all = wk.tile([P, NTMAX * HID], BF, name="xl_all",
                                     tag="xl_all", bufs=3)
                    for t0 in range(0, NT, 8):
                        ntc = min(8, NT - t0)
                        nc.gpsimd.dma_gather(
                            xl_all[:, t0 * HID:(t0 + ntc) * HID]
                                .rearrange("p (t c) -> p t c", c=HID),
                            xl_full[l][:],
                            idx_sb[l][:, idx_off[s] + t0 * 8:
                                      idx_off[s] + (t0 + ntc) * 8],
                            ntc * P, ntc * P, HID,
                            queue_num=1 + ((s + t0 // 8) % 3))
                    lg = wk.tile([P, NTMAX * 4], FP, name="lg", tag="lg", bufs=2)
                    lj = wk.tile([P, NTMAX], FP, name="lj", tag="lj", bufs=2)
                    for t in range(NT):
                        psw = ps_w.tile([P, HID], FP, space="PSUM",
                                        name="psw", tag="psw")
                        nc.tensor.matmul(out=psw[:],
                                         lhsT=mT_sb[:, t * P:(t + 1) * P],
                                         rhs=XRb[l % 2][:, s * HID:(s + 1) * HID],
                                         start=True, stop=False)
                        nc.tensor.matmul(out=psw[:], lhsT=ident_b[:],
                                         rhs=xl_all[:, t * HID:(t + 1) * HID],
                                         start=False, stop=True)
                        lk = wk.tile([P, HID], BF, name="lk", tag="lk", bufs=4)
                        slk = wk.tile([P, HID], BF, name="slk", tag="slk", bufs=4)
                        if H == 4:
                            nc.scalar.activation(out=lk[:], in_=psw[:],
                                                 func=AF.Prelu, alpha=NEG)
                            nc.vector.scalar_tensor_tensor(
                                out=slk[:], in0=lk[:], scalar=0.0,
                                in1=sgn_sb[l][:],
                                op0=ALU.bypass, op1=ALU.mult,
                                accum_out=lj[:, t:t + 1])
                            nc.vector.tensor_reduce(
                                out=lg[:, 4 * t:4 * t + 4],
                                in_=slk[:].rearrange("p (h d) -> p h d", h=4),
                                axis=AX.X, op=ALU.add)
                        else:
                            nc.scalar.activation(out=lk[:], in_=psw[:],
                                                 func=AF.Prelu, alpha=NEG)
                            nc.vector.scalar_tensor_tensor(
                                out=slk[:], in0=lk[:], scalar=0.0,
                                in1=sgn_sb[l][:],
                                op0=ALU.bypass, op1=ALU.mult,
                                accum_out=lg[:, t:t + 1])
                    ps_o = ps_o_pool.tile([P, 264], FP, space="PSUM",
                                          name="ps_o", tag="ps_o")
                    # EXP writes the (bf16) alpha columns of v_slot directly;
                    # the per-tile multiplies read them back per tile.
                    if H == 4:
                        v_slot = wk.tile([P, NTMAX * 260], BF, name="v_slot",
                                         tag="v_slot", bufs=2)
                        nc.scalar.activation(
                            out=v_slot[:, :NT * 260]
                                .rearrange("p (t c) -> p t c", c=260)[:, :, HID:260],
                            in_=lg[:, :NT * 4].rearrange("p (t h) -> p t h", h=4),
                            func=AF.Exp)
                        for t in range(NT):
                            nc.vector.tensor_tensor(
                                out=v_slot[:, t * 260:t * 260 + HID]
                                    .rearrange("p (h d) -> p h d", h=4),
                                in0=xl_all[:, t * HID:(t + 1) * HID]
                                    .rearrange("p (h d) -> p h d", h=4),
                                in1=v_slot[:, t * 260 + HID:t * 260 + HID + 4, None]
                                    .to_broadcast([P, 4, DIM]),
                                op=ALU.mult)
                            nc.tensor.matmul(out=ps_o[:, :260],
                                             lhsT=m_sb[:, t * P:(t + 1) * P],
                                             rhs=v_slot[:, t * 260:(t + 1) * 260],
                                             start=(t == 0), stop=(t == NT - 1))
                    else:
                        v_slot = wk.tile([P, NTMAX * 257], BF, name="v_slot2",
                                         tag="v_slot2", bufs=2)
                        nc.scalar.activation(
                            out=v_slot[:, :NT * 257]
                                .rearrange("p (t c) -> p t c", c=257)[:, :, HID:],
                            in_=lg[:, :NT].rearrange("p (t o) -> p t o", o=1),
                            func=AF.Exp)
                        al = wk.tile([P, NTMAX], FP, name="al", tag="al", bufs=2)
                        nc.scalar.activation(out=al[:, :NT], in_=lg[:, :NT],
                                             func=AF.Exp)
                        for t in range(NT):
                            nc.vector.tensor_scalar(
                                out=v_slot[:, t * 257:t * 257 + HID],
                                in0=xl_all[:, t * HID:(t + 1) * HID],
                                scalar1=al[:, t:t + 1],
                                scalar2=None, op0=ALU.mult)
                            nc.tensor.matmul(out=ps_o[:, :HID + 1],
                                             lhsT=m_sb[:, t * P:(t + 1) * P],
                                             rhs=v_slot[:, t * 257:(t + 1) * 257],
                                             start=(t == 0), stop=(t == NT - 1))
                    # finalize
                    g_sb = wk.tile([P, HID], BF, name="g_sb", tag="g_sb", bufs=2)
                    if H == 4:
                        rec = wk.tile([P, 4], FP, name="rec", tag="rec", bufs=2)
                        nc.vector.reciprocal(out=rec[:], in_=ps_o[:, HID:HID + 4])
                        gpre = wk.tile([P, HID], BF, name="gpre", tag="gpre",
                                       bufs=2)
                        nc.vector.tensor_tensor(
                            out=gpre[:].rearrange("p (h d) -> p h d", h=4),
                            in0=ps_o[:, :HID].rearrange("p (h d) -> p h d", h=4),
                            in1=rec[:, :, None].to_broadcast([P, 4, DIM]),
                            op=ALU.mult)
                        nc.scalar.activation(out=g_sb[:], in_=gpre[:], func=AF.Relu)
                    else:
                        rec = wk.tile([P, 1], FP, name="rec", tag="rec", bufs=2)
                        nc.vector.reciprocal(out=rec[:], in_=ps_o[:, HID:HID + 1])
                        nc.scalar.activation(out=g_sb[:], in_=ps_o[:, :HID],
                                             func=AF.Relu, scale=rec[:, 0:1])
                    nsl = slice(s * P, (s + 1) * P)
                    nexts = (next0, next1)
                    prevs = (prev0, prev1)
                    tp = ps_tp.tile([P, 2 * P], BF, space="PSUM",
                                    name="tp", tag="tp")
                    for k in range(2):
                        nc.tensor.transpose(out=tp[:, k * P:(k + 1) * P],
                                            in_=g_sb[:, k * P:(k + 1) * P],
                                            identity=ident_b[:])
                    for k in range(2):
                        nc.vector.scalar_tensor_tensor(
                            out=nexts[k][:, nsl], in0=tp[:, k * P:(k + 1) * P],
                            scalar=dinv_sb[l][:, k:k + 1],
                            in1=prevs[k][:, nsl],
                            op0=ALU.mult, op1=ALU.add)
                    if l < 3:
                        xlxr_slot(l + 1, s, next0, next1)
                        ag(l + 1, s)
                    elif s % 4 == 3:
                        phase_c_group(s // 4)

            # MLP head for one 512-node group; interleaved into layer 3's
            # slot loop (group j is ready once slots 4j..4j+3 finalize)
            def phase_c_group(j):
                sl = slice(j * 512, (j + 1) * 512)
                pm1 = ps_mm.tile([P, 512], FP, space="PSUM", name="pm1", tag="pmm")
                for k in range(2):
                    nc.tensor.matmul(out=pm1[:DIM, :],
                                     lhsT=Wm1_sb[:, k * DIM:(k + 1) * DIM],
                                     rhs=sT[k][:, sl], start=(k == 0), stop=False)
                nc.tensor.matmul(out=pm1[:DIM, :], lhsT=bm1_sb[:], rhs=ones_row[:],
                                 start=False, stop=True)
                nc.scalar.activation(out=m1T[:DIM, sl], in_=pm1[:DIM, :], func=AF.Relu)
                pm2 = ps_mm.tile([P, 512], FP, space="PSUM", name="pm2", tag="pmm")
                nc.tensor.matmul(out=pm2[:DIM, :], lhsT=Wm2_sb[:], rhs=m1T[:DIM, sl],
                                 start=True, stop=False)
                nc.tensor.matmul(out=pm2[:DIM, :], lhsT=bm2_sb[:], rhs=ones_row[:],
                                 start=False, stop=True)
                nc.scalar.activation(out=m2T[:DIM, sl], in_=pm2[:DIM, :], func=AF.Relu)
                py = ps_mm.tile([P, 512], FP, space="PSUM", name="py", tag="pmm")
                nc.tensor.matmul(out=py[:1, :], lhsT=Wm3_sb[:], rhs=m2T[:DIM, sl],
                                 start=True, stop=False)
                nc.tensor.matmul(out=py[:1, :], lhsT=bm3_sb[:], rhs=ones_row[:],
                                 start=False, stop=True)
                nc.scalar.activation(out=y_sb[:, sl], in_=py[:1, :], func=AF.Copy)
                nc.sync.dma_start(out=out[:, sl], in_=y_sb[:, sl])

            edge_layer(1, 4, sT[0], sT[1], aT[0], aT[1])
            edge_layer(2, 1, aT[0], aT[1], bT[0], bT[1])
            edge_layer(3, 1, bT[0], bT[1], sT[0], sT[1])

    nc.compile()
    return nc


_BUILD_CACHE = {}


def _get_program(key):
    if key not in _BUILD_CACHE:
        _BUILD_CACHE[key] = _build(key)
    return _BUILD_CACHE[key]


def kernel(**inputs) -> np.ndarray:
    global LAST_RESULTS
    ii = {k: np.asarray(v) for k, v in inputs.items()}
    assert ii["x"].shape == (N, F_IN)
    for l in (1, 2, 3):
        assert not np.any(ii[f"b{l}"]), "GAT bias assumed zero"

    idx1_all, idx23_all, m_host, mT_host, NT_slots, node_order = _prep_edges(
        np.asarray(ii["edge_index"], np.int64))
    w = _prep_weights(ii)

    def bf(a):
        return np.asarray(a, np.float32).astype(BF_NP)

    common = dict(
        Win=bf(ii["Win"]), b_in=bf(ii["b_in"])[None, :],
        Wskip=bf(ii["Wskip"]), bskip=bf(ii["bskip"])[None, :],
        WL1=w["WL1"], WR1=w["WR1"], WL2=w["WL2"], WR2=w["WR2"],
        WL3=w["WL3"], WR3=w["WR3"],
        sgn1=w["sgn1"], sgn2=w["sgn2"], sgn3=w["sgn3"],
        dinv1=w["dinv1"], dinv2=w["dinv2"], dinv3=w["dinv3"],
        Wm1=bf(ii["Wm1"]), bm1=bf(ii["bm1"])[None, :],
        Wm2=bf(ii["Wm2"]), bm2=bf(ii["bm2"])[None, :],
        Wm3=bf(ii["Wm3"]), bm3=bf(ii["bm3"])[None, :],
    )
    x = np.asarray(ii["x"], np.float32)
    in_maps = []
    for c in range(NCORES):
        m = dict(common)
        m["xT"] = np.ascontiguousarray(x[node_order[c]].T).astype(BF_NP)
        m["idx1"] = idx1_all[c]
        m["idx23"] = idx23_all[c]
        m["m_dram"] = m_host[c]
        m["mT_dram"] = mT_host[c]
        in_maps.append(m)

    nc = _get_program(NT_slots)
    res = run_bass_kernel_spmd(nc, in_maps, list(range(NCORES)),
                               trace=bool(os.environ.get("GAT_TRACE")))
    LAST_RESULTS = res
    y = np.empty(N, np.float32)
    for c in range(NCORES):
        y[node_order[c]] = res.results[c]["out"].reshape(-1)
    return y


# revision 50
# speedup vs baseline: 1.0527x; 1.0527x over previous
"""Trainium2 Bass kernel for the 3-layer GATv2 network (nn_GAT_35940286333219).

v4 (~935us vs 2111us v3 baseline): host-precomputed scatter masks in both
orientations streamed from DRAM (no on-device mask gen / PE transposes /
PSUM->SBUF mask copies); w = xl[src] + xr[dst] formed entirely on the PE as
two accumulating matmuls (mask-transpose gather + identity add); leaky-relu
via Prelu on the scalar engine; logits via one scalar_tensor_tensor
sign-multiply with free DVE accumulator (plus a per-head tensor_reduce for
the 4-head layer); |att| column scales folded into the tables with the
positive unscale applied past the ReLU inside the transposed residual add
(scalar_tensor_tensor from the transpose PSUM, no perm needed); alpha column
batched per slot into a [*, 257]-stride value buffer; per-slot gathers kept
at <=1024 descriptors (larger single gathers crash SWDGE); layer-1 table
slot-major with one AllGather, layers 2/3 chunk-major with a 12+4-slot
chunked AllGather (AG_CH) overlapping the xl/xr prep. Note: overlapping
next-layer gathers into the tail collective (src-chunk-sorted split gathers)
REGRESSED 25% - concurrent DMA-to-SBUF writes slow every DVE op ~30%.
"""
import os
import numpy as np
import ml_dtypes

import concourse.bacc as bacc
import concourse.bass as bass
import concourse.mybir as mybir
import concourse.tile as tile
from concourse.bass_utils import run_bass_kernel_spmd
from concourse.masks import make_identity

P = 128
N = 16384
NCORES = 8
NLOC = N // NCORES          # 2048
NBLK = NLOC // P            # 16 slots per core
F_IN = 128
DIM = 64
HID = 256
FP = mybir.dt.float32
BF = mybir.dt.bfloat16
I16 = mybir.dt.int16
AF = mybir.ActivationFunctionType
ALU = mybir.AluOpType
AX = mybir.AxisListType
BF_NP = ml_dtypes.bfloat16
NEG = 0.2
AG_CH = (0, 14, 16)         # AllGather chunk boundaries (slots)

LAST_RESULTS = None


def _prep_edges(edge_index):
    """Sort edges by dst, group per dst-block, per-core sort blocks by size
    (slot order), pad each slot to the max tile count across cores.

    Returns per-core idx arrays (int16, dma_gather wrap layout), host-built
    scatter masks in both orientations, NT per slot, table row map, and the
    local node order per core."""
    src = np.concatenate([edge_index[0], np.arange(N, dtype=np.int64)])
    dst = np.concatenate([edge_index[1], np.arange(N, dtype=np.int64)])
    order = np.argsort(dst, kind="stable")
    src_s, dst_s = src[order], dst[order]
    blk = dst_s // P                                  # global block 0..127
    bc = np.bincount(blk, minlength=NCORES * NBLK)
    starts = np.concatenate([[0], np.cumsum(bc)])

    # per-core slot order: blocks sorted by count desc
    slot_blocks = np.empty((NCORES, NBLK), np.int64)   # slot -> local block
    for c in range(NCORES):
        cnt = bc[c * NBLK:(c + 1) * NBLK]
        slot_blocks[c] = np.argsort(-cnt, kind="stable")
    # uniform tile count per slot = max over cores
    NT_slots = []
    for s in range(NBLK):
        ks = [bc[c * NBLK + slot_blocks[c, s]] for c in range(NCORES)]
        NT_slots.append(int(np.ceil(max(ks) / P)))
    EBLK = [nt * P for nt in NT_slots]

    # table row maps: node -> row in xl_full. Layer 1's table is slot-major
    # (one AllGather at startup); layers 2/3 are chunk-major (boundaries
    # AG_CH in slots) so their AllGathers run in chunks overlapping the
    # per-slot xl/xr prep.
    rownum1 = np.empty(N, np.int64)
    rownum23 = np.empty(N, np.int64)
    node_order = np.empty((NCORES, NLOC), np.int64)    # local idx -> node id
    for c in range(NCORES):
        for s in range(NBLK):
            b = slot_blocks[c, s]
            nodes = c * NLOC + b * P + np.arange(P)
            rownum1[nodes] = c * NLOC + s * P + np.arange(P)
            q = sum(1 for ch in AG_CH[1:-1] if s >= ch)
            pref = AG_CH[q] * P
            crows = (AG_CH[q + 1] - AG_CH[q]) * P
            rownum23[nodes] = (pref * NCORES + c * crows
                               + (s - AG_CH[q]) * P + np.arange(P))
            node_order[c, s * P:(s + 1) * P] = nodes

    tot_idx_cols = sum(e // 16 for e in EBLK)
    tot_nt = sum(NT_slots)
    idx1_all = np.zeros((NCORES, P, tot_idx_cols), np.int16)
    idx23_all = np.zeros((NCORES, P, tot_idx_cols), np.int16)
    m_host = np.zeros((NCORES, P, tot_nt * P), BF_NP)
    mT_host = np.zeros((NCORES, P, tot_nt * P), BF_NP)
    eye129 = np.zeros((P + 1, P), np.float32)
    eye129[np.arange(P), np.arange(P)] = 1.0
    for c in range(NCORES):
        ioff = noff = 0
        for s in range(NBLK):
            b = slot_blocks[c, s]
            g = c * NBLK + b
            lo, hi = starts[g], starts[g + 1]
            k = hi - lo
            e = EBLK[s]
            nt = NT_slots[s]
            for rn, ia in ((rownum1, idx1_all), (rownum23, idx23_all)):
                srcrow = np.zeros(e, np.int64)         # pads gather row 0
                srcrow[:k] = rn[src_s[lo:hi]]
                wrapped = srcrow.reshape(e // 16, 16).T
                ia[c, :, ioff:ioff + e // 16] = np.tile(wrapped, (8, 1))
            dcol = np.full(e, P, np.int64)             # pad -> zero mask row
            dcol[:k] = dst_s[lo:hi] - g * P
            oh = eye129[dcol]                          # [e, P]
            oh3 = oh.reshape(nt, P, P)
            m_host[c, :, noff * P:(noff + nt) * P] = (
                oh3.transpose(1, 0, 2).reshape(P, nt * P))
            mT_host[c, :, noff * P:(noff + nt) * P] = (
                oh3.transpose(2, 0, 1).reshape(P, nt * P))
            ioff += e // 16
            noff += nt
    return (idx1_all, idx23_all, m_host, mT_host, tuple(NT_slots), node_order)


def _prep_weights(ii):
    """Fold |att| into the Wl/Wr columns (positive scale); keep the sign in a
    broadcast row for the logit reduce, and the reciprocal scale (applied past
    the ReLU) for the transposed residual add."""
    out = {}
    for l in (1, 2, 3):
        Wl = np.asarray(ii[f"Wl{l}"], np.float32)
        Wr = np.asarray(ii[f"Wr{l}"], np.float32)
        att = np.asarray(ii[f"att{l}"], np.float32).reshape(-1)
        sc = np.maximum(np.abs(att), 1e-6)
        out[f"WL{l}"] = (Wl * sc[None, :]).astype(BF_NP)
        out[f"WR{l}"] = (Wr * sc[None, :]).astype(BF_NP)
        out[f"sgn{l}"] = np.tile(np.sign(att)[None, :].astype(BF_NP), (P, 1))
        out[f"dinv{l}"] = np.ascontiguousarray(
            (1.0 / sc).reshape(2, P).T.astype(np.float32))   # [P, 2]
    return out


def _build(NT_slots):
    nc = bacc.Bacc(None, num_swdge_queues=4)
    EBLK = [nt * P for nt in NT_slots]
    NTMAX = max(NT_slots)
    tot_idx_cols = sum(e // 16 for e in EBLK)
    tot_nt = sum(NT_slots)

    def par(name, shape, dtype=BF):
        return nc.declare_dram_parameter(name, list(shape), dtype, isOutput=False)

    xT = par("xT", [F_IN, NLOC])
    idx1 = par("idx1", [P, tot_idx_cols], I16)
    idx23 = par("idx23", [P, tot_idx_cols], I16)
    m_dram = par("m_dram", [P, tot_nt * P])
    mT_dram = par("mT_dram", [P, tot_nt * P])
    Win = par("Win", [F_IN, DIM]); b_in = par("b_in", [1, DIM])
    Wskip = par("Wskip", [DIM, HID]); bskip = par("bskip", [1, HID])
    WL1 = par("WL1", [DIM, HID]); WR1 = par("WR1", [DIM, HID])
    WL2 = par("WL2", [HID, HID]); WR2 = par("WR2", [HID, HID])
    WL3 = par("WL3", [HID, HID]); WR3 = par("WR3", [HID, HID])
    sgn = {l: par(f"sgn{l}", [P, HID]) for l in (1, 2, 3)}
    dinv = {l: par(f"dinv{l}", [P, 2], FP) for l in (1, 2, 3)}
    Wm1 = par("Wm1", [HID, DIM]); bm1 = par("bm1", [1, DIM])
    Wm2 = par("Wm2", [DIM, DIM]); bm2 = par("bm2", [1, DIM])
    Wm3 = par("Wm3", [DIM, 1]); bm3 = par("bm3", [1, 1])
    out = nc.declare_dram_parameter("out", [1, NLOC], FP, isOutput=True)

    xl_loc = {l: nc.dram_tensor(f"xl_loc{l}", [NLOC, HID], BF) for l in (1, 2, 3)}
    xl_full = {l: nc.dram_tensor(f"xl_full{l}", [N, HID], BF, addr_space="Shared")
               for l in (1, 2, 3)}

    with tile.TileContext(nc) as tc:
        with (
            tc.tile_pool(name="const", bufs=1) as cp,
            tc.tile_pool(name="big", bufs=1) as bigp,
            tc.tile_pool(name="wk", bufs=1) as wk,
            tc.tile_pool(name="ps_mm", bufs=2, space="PSUM") as ps_mm,
            tc.tile_pool(name="ps_w", bufs=3, space="PSUM") as ps_w,
            tc.tile_pool(name="ps_o", bufs=2, space="PSUM") as ps_o_pool,
            tc.tile_pool(name="ps_tp", bufs=1, space="PSUM") as ps_tp,
        ):
            def load_const(pname, ap, shape, dtype=BF):
                t = cp.tile(list(shape), dtype, name=pname + "_sb")
                nc.sync.dma_start(out=t[:], in_=ap[:])
                return t

            def load_const_2k(pname, ap, cols):
                t = cp.tile([P, 2 * cols], BF, name=pname + "_sb")
                nc.sync.dma_start(out=t[:, :cols], in_=ap[:P, :])
                nc.sync.dma_start(out=t[:, cols:], in_=ap[P:, :])
                return t

            ident_f = cp.tile([P, P], FP, name="ident_f")
            make_identity(nc, ident_f[:])
            ident_b = cp.tile([P, P], BF, name="ident_b")
            nc.vector.tensor_copy(out=ident_b[:], in_=ident_f[:])
            ones_row = cp.tile([1, 512], BF, name="ones_row")
            nc.vector.memset(ones_row[:], 1.0)
            ones_col = cp.tile([P, 1], BF, name="ones_col")
            nc.vector.memset(ones_col[:], 1.0)

            xT_sb = load_const("xT", xT, [F_IN, NLOC])
            idx_sb = {1: load_const("idx1", idx1, [P, tot_idx_cols], I16),
                      2: load_const("idx23", idx23, [P, tot_idx_cols], I16)}
            idx_sb[3] = idx_sb[2]
            Win_sb = load_const("Win", Win, [F_IN, DIM])
            b_in_sb = load_const("b_in", b_in, [1, DIM])
            Wskip_sb = load_const("Wskip", Wskip, [DIM, HID])
            bskip_sb = load_const("bskip", bskip, [1, HID])
            WL_sb = {1: load_const("WL1", WL1, [DIM, HID]),
                     2: load_const_2k("WL2", WL2, HID),
                     3: load_const_2k("WL3", WL3, HID)}
            WR_sb = {1: load_const("WR1", WR1, [DIM, HID]),
                     2: load_const_2k("WR2", WR2, HID),
                     3: load_const_2k("WR3", WR3, HID)}
            sgn_sb = {l: load_const(f"sgn{l}", sgn[l], [P, HID])
                      for l in (1, 2, 3)}
            dinv_sb = {l: load_const(f"dinv{l}", dinv[l], [P, 2], FP)
                       for l in (1, 2, 3)}
            Wm1_sb = load_const_2k("Wm1", Wm1, DIM)
            bm1_sb = load_const("bm1", bm1, [1, DIM])
            Wm2_sb = load_const("Wm2", Wm2, [DIM, DIM])
            bm2_sb = load_const("bm2", bm2, [1, DIM])
            Wm3_sb = load_const("Wm3", Wm3, [DIM, 1])
            bm3_sb = load_const("bm3", bm3, [1, 1])

            # transposed residual stream buffers (feature chunk k on partitions)
            sT = [bigp.tile([P, NLOC], BF, name=f"sT{k}") for k in range(2)]
            aT = [bigp.tile([P, NLOC], BF, name=f"aT{k}") for k in range(2)]
            bT = [bigp.tile([P, NLOC], BF, name=f"bT{k}") for k in range(2)]
            XRb = {0: bigp.tile([P, NBLK * HID], BF, name="XRb0"),
                   1: bigp.tile([P, NBLK * HID], BF, name="XRb1")}
            hT = bigp.tile([DIM, NLOC], BF, name="hT")
            m1T = bigp.tile([DIM, NLOC], BF, name="m1T")
            m2T = bigp.tile([DIM, NLOC], BF, name="m2T")
            y_sb = bigp.tile([1, NLOC], FP, name="y_sb")

            idx_off = [0]
            nt_off = [0]
            for s in range(NBLK):
                idx_off.append(idx_off[-1] + EBLK[s] // 16)
                nt_off.append(nt_off[-1] + NT_slots[s])

            def ag(l, s):
                if l == 1:                 # slot-major table, one collective
                    if s == NBLK - 1:
                        nc.gpsimd.collective_compute(
                            "AllGather", ALU.bypass,
                            replica_groups=[list(range(NCORES))],
                            ins=[xl_loc[1][:]], outs=[xl_full[1][:]])
                    return
                if s + 1 not in AG_CH:
                    return
                q = AG_CH.index(s + 1) - 1
                lo, hi = AG_CH[q] * P, AG_CH[q + 1] * P
                nc.gpsimd.collective_compute(
                    "AllGather", ALU.bypass,
                    replica_groups=[list(range(NCORES))],
                    ins=[xl_loc[l][lo:hi, :]],
                    outs=[xl_full[l][lo * NCORES:hi * NCORES, :]])

            def xlxr_slot(l, s, src0, src1):
                """xl/xr for layer l, slot s, from actT chunks src0/src1
                (or hT when l == 1); stores xl row-block, fills XRb."""
                nsl = slice(s * P, (s + 1) * P)
                pxl = ps_mm.tile([P, HID], FP, space="PSUM", name="pxl", tag="pmm")
                pxr = ps_mm.tile([P, HID], FP, space="PSUM", name="pxr", tag="pmm")
                if l == 1:
                    nc.tensor.matmul(out=pxl[:], lhsT=hT[:DIM, nsl], rhs=WL_sb[1][:],
                                     start=True, stop=True)
                    nc.tensor.matmul(out=pxr[:], lhsT=hT[:DIM, nsl], rhs=WR_sb[1][:],
                                     start=True, stop=True)
                else:
                    srcs = (src0, src1)
                    for k in range(2):
                        nc.tensor.matmul(out=pxl[:], lhsT=srcs[k][:, nsl],
                                         rhs=WL_sb[l][:, k * HID:(k + 1) * HID],
                                         start=(k == 0), stop=(k == 1))
                    for k in range(2):
                        nc.tensor.matmul(out=pxr[:], lhsT=srcs[k][:, nsl],
                                         rhs=WR_sb[l][:, k * HID:(k + 1) * HID],
                                         start=(k == 0), stop=(k == 1))
                xst = wk.tile([P, HID], BF, name="xst", tag="xst", bufs=4)
                nc.scalar.activation(out=xst[:], in_=pxl[:], func=AF.Copy)
                nc.sync.dma_start(out=xl_loc[l][nsl, :], in_=xst[:])
                nc.scalar.activation(out=XRb[l % 2][:, s * HID:(s + 1) * HID],
                                     in_=pxr[:], func=AF.Copy)

            # ---------------- phase A ----------------
            for j in range(NLOC // 512):
                sl = slice(j * 512, (j + 1) * 512)
                pmm = ps_mm.tile([P, 512], FP, space="PSUM", name="pmm", tag="pmm")
                nc.tensor.matmul(out=pmm[:DIM, :], lhsT=Win_sb[:], rhs=xT_sb[:, sl],
                                 start=True, stop=False)
                nc.tensor.matmul(out=pmm[:DIM, :], lhsT=b_in_sb[:], rhs=ones_row[:],
                                 start=False, stop=True)
                nc.scalar.activation(out=hT[:DIM, sl], in_=pmm[:DIM, :], func=AF.Relu)

            for s in range(NBLK):
                xlxr_slot(1, s, None, None)
                ag(1, s)

            # skip projection overlaps the layer-1 AllGather
            for k in range(2):
                ksl = slice(k * P, (k + 1) * P)
                for j in range(NLOC // 512):
                    sl = slice(j * 512, (j + 1) * 512)
                    psk = ps_mm.tile([P, 512], FP, space="PSUM", name="psk", tag="pmm")
                    nc.tensor.matmul(out=psk[:], lhsT=Wskip_sb[:, ksl],
                                     rhs=hT[:DIM, sl], start=True, stop=False)
                    nc.tensor.matmul(out=psk[:], lhsT=bskip_sb[:, ksl],
                                     rhs=ones_row[:], start=False, stop=True)
                    nc.scalar.activation(out=sT[k][:, sl], in_=psk[:], func=AF.Copy)

            # ---------------- edge stage ----------------
            def edge_layer(l, H, prev0, prev1, next0, next1):
                for s in range(NBLK):
                    NT = NT_slots[s]
                    m_sb = wk.tile([P, NTMAX * P], BF, name="m_sb",
                                   tag="m_sb", bufs=3)
                    nc.sync.dma_start(
                        out=m_sb[:, :NT * P],
                        in_=m_dram[:, nt_off[s] * P:nt_off[s + 1] * P])
                    mT_sb = wk.tile([P, NTMAX * P], BF, name="mT_sb",
                                    tag="mT_sb", bufs=3)
                    nc.sync.dma_start(
                        out=mT_sb[:, :NT * P],
                        in_=mT_dram[:, nt_off[s] * P:nt_off[s + 1] * P])
                    xl_all = wk.tile([P, NTMAX * HID], BF, name="xl_all",
                                     tag="xl_all", bufs=3)
                    for t0 in range(0, NT, 8):
                        ntc = min(8, NT - t0)
                        nc.gpsimd.dma_gather(
                            xl_all[:, t0 * HID:(t0 + ntc) * HID]
                                .rearrange("p (t c) -> p t c", c=HID),
                            xl_full[l][:],
                            idx_sb[l][:, idx_off[s] + t0 * 8:
                                      idx_off[s] + (t0 + ntc) * 8],
                            ntc * P, ntc * P, HID,
                            queue_num=1 + ((s + t0 // 8) % 3))
                    lg = wk.tile([P, NTMAX * 4], FP, name="lg", tag="lg", bufs=2)
                    lj = wk.tile([P, NTMAX], FP, name="lj", tag="lj", bufs=2)
                    for t in range(NT):
                        psw = ps_w.tile([P, HID], FP, space="PSUM",
                                        name="psw", tag="psw")
                        nc.tensor.matmul(out=psw[:],
                                         lhsT=mT_sb[:, t * P:(t + 1) * P],
                                         rhs=XRb[l % 2][:, s * HID:(s + 1) * HID],
                                         start=True, stop=False)
                        nc.tensor.matmul(out=psw[:], lhsT=ident_b[:],
                                         rhs=xl_all[:, t * HID:(t + 1) * HID],
                                         start=False, stop=True)
                        lk = wk.tile([P, HID], BF, name="lk", tag="lk", bufs=4)
                        slk = wk.tile([P, HID], BF, name="slk", tag="slk", bufs=4)
                        if H == 4:
                            nc.scalar.activation(out=lk[:], in_=psw[:],
                                                 func=AF.Prelu, alpha=NEG)
                            nc.vector.scalar_tensor_tensor(
                                out=slk[:], in0=lk[:], scalar=0.0,
                                in1=sgn_sb[l][:],
                                op0=ALU.bypass, op1=ALU.mult,
                                accum_out=lj[:, t:t + 1])
                            nc.vector.tensor_reduce(
                                out=lg[:, 4 * t:4 * t + 4],
                                in_=slk[:].rearrange("p (h d) -> p h d", h=4),
                                axis=AX.X, op=ALU.add)
                        else:
                            nc.scalar.activation(out=lk[:], in_=psw[:],
                                                 func=AF.Prelu, alpha=NEG)
                            nc.vector.scalar_tensor_tensor(
                                out=slk[:], in0=lk[:], scalar=0.0,
                                in1=sgn_sb[l][:],
                                op0=ALU.bypass, op1=ALU.mult,
                                accum_out=lg[:, t:t + 1])
                    al = wk.tile([P, NTMAX * 4], FP, name="al", tag="al", bufs=2)
                    nc.scalar.activation(out=al[:, :NT * H], in_=lg[:, :NT * H],
                                         func=AF.Exp)
                    ps_o = ps_o_pool.tile([P, 264], FP, space="PSUM",
                                          name="ps_o", tag="ps_o")
                    if H == 4:
                        v_slot = wk.tile([P, NTMAX * 260], BF, name="v_slot",
                                         tag="v_slot", bufs=2)
                        nc.vector.tensor_copy(
                            out=v_slot[:, :NT * 260]
                                .rearrange("p (t c) -> p t c", c=260)[:, :, HID:260],
                            in_=al[:, :NT * 4].rearrange("p (t h) -> p t h", h=4))
                        for t in range(NT):
                            nc.vector.tensor_tensor(
                                out=v_slot[:, t * 260:t * 260 + HID]
                                    .rearrange("p (h d) -> p h d", h=4),
                                in0=xl_all[:, t * HID:(t + 1) * HID]
                                    .rearrange("p (h d) -> p h d", h=4),
                                in1=al[:, 4 * t:4 * t + 4, None]
                                    .to_broadcast([P, 4, DIM]),
                                op=ALU.mult)
                            nc.tensor.matmul(out=ps_o[:, :260],
                                             lhsT=m_sb[:, t * P:(t + 1) * P],
                                             rhs=v_slot[:, t * 260:(t + 1) * 260],
                                             start=(t == 0), stop=(t == NT - 1))
                    else:
                        v_slot = wk.tile([P, NTMAX * 257], BF, name="v_slot2",
                                         tag="v_slot2", bufs=2)
                        nc.vector.tensor_copy(
                            out=v_slot[:, :NT * 257]
                                .rearrange("p (t c) -> p t c", c=257)[:, :, HID:],
                            in_=al[:, :NT, None])
                        for t in range(NT):
                            nc.vector.tensor_scalar(
                                out=v_slot[:, t * 257:t * 257 + HID],
                                in0=xl_all[:, t * HID:(t + 1) * HID],
                                scalar1=al[:, t:t + 1], scalar2=None,
                                op0=ALU.mult)
                            nc.tensor.matmul(out=ps_o[:, :HID + 1],
                                             lhsT=m_sb[:, t * P:(t + 1) * P],
                                             rhs=v_slot[:, t * 257:(t + 1) * 257],
                                             start=(t == 0), stop=(t == NT - 1))
                    # finalize
                    g_sb = wk.tile([P, HID], BF, name="g_sb", tag="g_sb", bufs=2)
                    if H == 4:
                        rec = wk.tile([P, 4], FP, name="rec", tag="rec", bufs=2)
                        nc.vector.reciprocal(out=rec[:], in_=ps_o[:, HID:HID + 4])
                        gpre = wk.tile([P, HID], BF, name="gpre", tag="gpre",
                                       bufs=2)
                        nc.vector.tensor_tensor(
                            out=gpre[:].rearrange("p (h d) -> p h d", h=4),
                            in0=ps_o[:, :HID].rearrange("p (h d) -> p h d", h=4),
                            in1=rec[:, :, None].to_broadcast([P, 4, DIM]),
                            op=ALU.mult)
                        nc.scalar.activation(out=g_sb[:], in_=gpre[:], func=AF.Relu)
                    else:
                        rec = wk.tile([P, 1], FP, name="rec", tag="rec", bufs=2)
                        nc.vector.reciprocal(out=rec[:], in_=ps_o[:, HID:HID + 1])
                        nc.scalar.activation(out=g_sb[:], in_=ps_o[:, :HID],
                                             func=AF.Relu, scale=rec[:, 0:1])
                    nsl = slice(s * P, (s + 1) * P)
                    nexts = (next0, next1)
                    prevs = (prev0, prev1)
                    tp = ps_tp.tile([P, 2 * P], BF, space="PSUM",
                                    name="tp", tag="tp")
                    for k in range(2):
                        nc.tensor.transpose(out=tp[:, k * P:(k + 1) * P],
                                            in_=g_sb[:, k * P:(k + 1) * P],
                                            identity=ident_b[:])
                    for k in range(2):
                        nc.vector.scalar_tensor_tensor(
                            out=nexts[k][:, nsl], in0=tp[:, k * P:(k + 1) * P],
                            scalar=dinv_sb[l][:, k:k + 1],
                            in1=prevs[k][:, nsl],
                            op0=ALU.mult, op1=ALU.add)
                    if l < 3:
                        xlxr_slot(l + 1, s, next0, next1)
                        ag(l + 1, s)
                    elif s % 4 == 3:
                        phase_c_group(s // 4)

            # MLP head for one 512-node group; interleaved into layer 3's
            # slot loop (group j is ready once slots 4j..4j+3 finalize)
            def phase_c_group(j):
                sl = slice(j * 512, (j + 1) * 512)
                pm1 = ps_mm.tile([P, 512], FP, space="PSUM", name="pm1", tag="pmm")
                for k in range(2):
                    nc.tensor.matmul(out=pm1[:DIM, :],
                                     lhsT=Wm1_sb[:, k * DIM:(k + 1) * DIM],
                                     rhs=sT[k][:, sl], start=(k == 0), stop=False)
                nc.tensor.matmul(out=pm1[:DIM, :], lhsT=bm1_sb[:], rhs=ones_row[:],
                                 start=False, stop=True)
                nc.scalar.activation(out=m1T[:DIM, sl], in_=pm1[:DIM, :], func=AF.Relu)
                pm2 = ps_mm.tile([P, 512], FP, space="PSUM", name="pm2", tag="pmm")
                nc.tensor.matmul(out=pm2[:DIM, :], lhsT=Wm2_sb[:], rhs=m1T[:DIM, sl],
                                 start=True, stop=False)
                nc.tensor.matmul(out=pm2[:DIM, :], lhsT=bm2_sb[:], rhs=ones_row[:],
                                 start=False, stop=True)
                nc.scalar.activation(out=m2T[:DIM, sl], in_=pm2[:DIM, :], func=AF.Relu)
                py = ps_mm.tile([P, 512], FP, space="PSUM", name="py", tag="pmm")
                nc.tensor.matmul(out=py[:1, :], lhsT=Wm3_sb[:], rhs=m2T[:DIM, sl],
                                 start=True, stop=False)
                nc.tensor.matmul(out=py[:1, :], lhsT=bm3_sb[:], rhs=ones_row[:],
                                 start=False, stop=True)
                nc.scalar.activation(out=y_sb[:, sl], in_=py[:1, :], func=AF.Copy)
                nc.sync.dma_start(out=out[:, sl], in_=y_sb[:, sl])

            edge_layer(1, 4, sT[0], sT[1], aT[0], aT[1])
            edge_layer(2, 1, aT[0], aT[1], bT[0], bT[1])
            edge_layer(3, 1, bT[0], bT[1], sT[0], sT[1])

    nc.compile()
    return nc


_BUILD_CACHE = {}


def _get_program(key):
    if key not in _BUILD_CACHE:
        _BUILD_CACHE[key] = _build(key)
    return _BUILD_CACHE[key]


def kernel(**inputs) -> np.ndarray:
    global LAST_RESULTS
    ii = {k: np.asarray(v) for k, v in inputs.items()}
    assert ii["x"].shape == (N, F_IN)
    for l in (1, 2, 3):
        assert not np.any(ii[f"b{l}"]), "GAT bias assumed zero"

    idx1_all, idx23_all, m_host, mT_host, NT_slots, node_order = _prep_edges(
        np.asarray(ii["edge_index"], np.int64))
    w = _prep_weights(ii)

    def bf(a):
        return np.asarray(a, np.float32).astype(BF_NP)

    common = dict(
        Win=bf(ii["Win"]), b_in=bf(ii["b_in"])[None, :],
        Wskip=bf(ii["Wskip"]), bskip=bf(ii["bskip"])[None, :],
        WL1=w["WL1"], WR1=w["WR1"], WL2=w["WL2"], WR2=w["WR2"],
        WL3=w["WL3"], WR3=w["WR3"],
        sgn1=w["sgn1"], sgn2=w["sgn2"], sgn3=w["sgn3"],
        dinv1=w["dinv1"], dinv2=w["dinv2"], dinv3=w["dinv3"],
        Wm1=bf(ii["Wm1"]), bm1=bf(ii["bm1"])[None, :],
        Wm2=bf(ii["Wm2"]), bm2=bf(ii["bm2"])[None, :],
        Wm3=bf(ii["Wm3"]), bm3=bf(ii["bm3"])[None, :],
    )
    x = np.asarray(ii["x"], np.float32)
    in_maps = []
    for c in range(NCORES):
        m = dict(common)
        m["xT"] = np.ascontiguousarray(x[node_order[c]].T).astype(BF_NP)
        m["idx1"] = idx1_all[c]
        m["idx23"] = idx23_all[c]
        m["m_dram"] = m_host[c]
        m["mT_dram"] = mT_host[c]
        in_maps.append(m)

    nc = _get_program(NT_slots)
    res = run_bass_kernel_spmd(nc, in_maps, list(range(NCORES)),
                               trace=bool(os.environ.get("GAT_TRACE")))
    LAST_RESULTS = res
    y = np.empty(N, np.float32)
    for c in range(NCORES):
        y[node_order[c]] = res.results[c]["out"].reshape(-1)
    return y


# revision 51
# speedup vs baseline: 1.0554x; 1.0025x over previous
"""Trainium2 Bass kernel for the 3-layer GATv2 network (nn_GAT_35940286333219).

v4 (~935us vs 2111us v3 baseline): host-precomputed scatter masks in both
orientations streamed from DRAM (no on-device mask gen / PE transposes /
PSUM->SBUF mask copies); w = xl[src] + xr[dst] formed entirely on the PE as
two accumulating matmuls (mask-transpose gather + identity add); leaky-relu
via Prelu on the scalar engine; logits via one scalar_tensor_tensor
sign-multiply with free DVE accumulator (plus a per-head tensor_reduce for
the 4-head layer); |att| column scales folded into the tables with the
positive unscale applied past the ReLU inside the transposed residual add
(scalar_tensor_tensor from the transpose PSUM, no perm needed); alpha column
batched per slot into a [*, 257]-stride value buffer; per-slot gathers kept
at <=1024 descriptors (larger single gathers crash SWDGE); layer-1 table
slot-major with one AllGather, layers 2/3 chunk-major with a 12+4-slot
chunked AllGather (AG_CH) overlapping the xl/xr prep. Note: overlapping
next-layer gathers into the tail collective (src-chunk-sorted split gathers)
REGRESSED 25% - concurrent DMA-to-SBUF writes slow every DVE op ~30%.
"""
import os
import numpy as np
import ml_dtypes

import concourse.bacc as bacc
import concourse.bass as bass
import concourse.mybir as mybir
import concourse.tile as tile
from concourse.bass_utils import run_bass_kernel_spmd
from concourse.masks import make_identity

P = 128
N = 16384
NCORES = 8
NLOC = N // NCORES          # 2048
NBLK = NLOC // P            # 16 slots per core
F_IN = 128
DIM = 64
HID = 256
FP = mybir.dt.float32
BF = mybir.dt.bfloat16
I16 = mybir.dt.int16
AF = mybir.ActivationFunctionType
ALU = mybir.AluOpType
AX = mybir.AxisListType
BF_NP = ml_dtypes.bfloat16
NEG = 0.2
AG_CH = (0, 14, 16)         # AllGather chunk boundaries (slots)

LAST_RESULTS = None


def _prep_edges(edge_index):
    """Sort edges by dst, group per dst-block, per-core sort blocks by size
    (slot order), pad each slot to the max tile count across cores.

    Returns per-core idx arrays (int16, dma_gather wrap layout), host-built
    scatter masks in both orientations, NT per slot, table row map, and the
    local node order per core."""
    src = np.concatenate([edge_index[0], np.arange(N, dtype=np.int64)])
    dst = np.concatenate([edge_index[1], np.arange(N, dtype=np.int64)])
    order = np.argsort(dst, kind="stable")
    src_s, dst_s = src[order], dst[order]
    blk = dst_s // P                                  # global block 0..127
    bc = np.bincount(blk, minlength=NCORES * NBLK)
    starts = np.concatenate([[0], np.cumsum(bc)])

    # per-core slot order: blocks sorted by count desc
    slot_blocks = np.empty((NCORES, NBLK), np.int64)   # slot -> local block
    for c in range(NCORES):
        cnt = bc[c * NBLK:(c + 1) * NBLK]
        slot_blocks[c] = np.argsort(-cnt, kind="stable")
    # uniform tile count per slot = max over cores
    NT_slots = []
    for s in range(NBLK):
        ks = [bc[c * NBLK + slot_blocks[c, s]] for c in range(NCORES)]
        NT_slots.append(int(np.ceil(max(ks) / P)))
    EBLK = [nt * P for nt in NT_slots]

    # table row maps: node -> row in xl_full. Layer 1's table is slot-major
    # (one AllGather at startup); layers 2/3 are chunk-major (boundaries
    # AG_CH in slots) so their AllGathers run in chunks overlapping the
    # per-slot xl/xr prep.
    rownum1 = np.empty(N, np.int64)
    rownum23 = np.empty(N, np.int64)
    node_order = np.empty((NCORES, NLOC), np.int64)    # local idx -> node id
    for c in range(NCORES):
        for s in range(NBLK):
            b = slot_blocks[c, s]
            nodes = c * NLOC + b * P + np.arange(P)
            rownum1[nodes] = c * NLOC + s * P + np.arange(P)
            q = sum(1 for ch in AG_CH[1:-1] if s >= ch)
            pref = AG_CH[q] * P
            crows = (AG_CH[q + 1] - AG_CH[q]) * P
            rownum23[nodes] = (pref * NCORES + c * crows
                               + (s - AG_CH[q]) * P + np.arange(P))
            node_order[c, s * P:(s + 1) * P] = nodes

    tot_idx_cols = sum(e // 16 for e in EBLK)
    tot_nt = sum(NT_slots)
    idx1_all = np.zeros((NCORES, P, tot_idx_cols), np.int16)
    idx23_all = np.zeros((NCORES, P, tot_idx_cols), np.int16)
    m_host = np.zeros((NCORES, P, tot_nt * P), BF_NP)
    mT_host = np.zeros((NCORES, P, tot_nt * P), BF_NP)
    eye129 = np.zeros((P + 1, P), np.float32)
    eye129[np.arange(P), np.arange(P)] = 1.0
    for c in range(NCORES):
        ioff = noff = 0
        for s in range(NBLK):
            b = slot_blocks[c, s]
            g = c * NBLK + b
            lo, hi = starts[g], starts[g + 1]
            k = hi - lo
            e = EBLK[s]
            nt = NT_slots[s]
            for rn, ia in ((rownum1, idx1_all), (rownum23, idx23_all)):
                srcrow = np.zeros(e, np.int64)         # pads gather row 0
                srcrow[:k] = rn[src_s[lo:hi]]
                wrapped = srcrow.reshape(e // 16, 16).T
                ia[c, :, ioff:ioff + e // 16] = np.tile(wrapped, (8, 1))
            dcol = np.full(e, P, np.int64)             # pad -> zero mask row
            dcol[:k] = dst_s[lo:hi] - g * P
            oh = eye129[dcol]                          # [e, P]
            oh3 = oh.reshape(nt, P, P)
            m_host[c, :, noff * P:(noff + nt) * P] = (
                oh3.transpose(1, 0, 2).reshape(P, nt * P))
            mT_host[c, :, noff * P:(noff + nt) * P] = (
                oh3.transpose(2, 0, 1).reshape(P, nt * P))
            ioff += e // 16
            noff += nt
    return (idx1_all, idx23_all, m_host, mT_host, tuple(NT_slots), node_order)


def _prep_weights(ii):
    """Fold |att| into the Wl/Wr columns (positive scale); keep the sign in a
    broadcast row for the logit reduce, and the reciprocal scale (applied past
    the ReLU) for the transposed residual add."""
    out = {}
    for l in (1, 2, 3):
        Wl = np.asarray(ii[f"Wl{l}"], np.float32)
        Wr = np.asarray(ii[f"Wr{l}"], np.float32)
        att = np.asarray(ii[f"att{l}"], np.float32).reshape(-1)
        sc = np.maximum(np.abs(att), 1e-6)
        out[f"WL{l}"] = (Wl * sc[None, :]).astype(BF_NP)
        out[f"WR{l}"] = (Wr * sc[None, :]).astype(BF_NP)
        out[f"sgn{l}"] = np.tile(np.sign(att)[None, :].astype(BF_NP), (P, 1))
        out[f"dinv{l}"] = np.ascontiguousarray(
            (1.0 / sc).reshape(2, P).T.astype(np.float32))   # [P, 2]
    return out


def _build(NT_slots):
    nc = bacc.Bacc(None, num_swdge_queues=4)
    EBLK = [nt * P for nt in NT_slots]
    NTMAX = max(NT_slots)
    tot_idx_cols = sum(e // 16 for e in EBLK)
    tot_nt = sum(NT_slots)

    def par(name, shape, dtype=BF):
        return nc.declare_dram_parameter(name, list(shape), dtype, isOutput=False)

    xT = par("xT", [F_IN, NLOC])
    idx1 = par("idx1", [P, tot_idx_cols], I16)
    idx23 = par("idx23", [P, tot_idx_cols], I16)
    m_dram = par("m_dram", [P, tot_nt * P])
    mT_dram = par("mT_dram", [P, tot_nt * P])
    Win = par("Win", [F_IN, DIM]); b_in = par("b_in", [1, DIM])
    Wskip = par("Wskip", [DIM, HID]); bskip = par("bskip", [1, HID])
    WL1 = par("WL1", [DIM, HID]); WR1 = par("WR1", [DIM, HID])
    WL2 = par("WL2", [HID, HID]); WR2 = par("WR2", [HID, HID])
    WL3 = par("WL3", [HID, HID]); WR3 = par("WR3", [HID, HID])
    sgn = {l: par(f"sgn{l}", [P, HID]) for l in (1, 2, 3)}
    dinv = {l: par(f"dinv{l}", [P, 2], FP) for l in (1, 2, 3)}
    Wm1 = par("Wm1", [HID, DIM]); bm1 = par("bm1", [1, DIM])
    Wm2 = par("Wm2", [DIM, DIM]); bm2 = par("bm2", [1, DIM])
    Wm3 = par("Wm3", [DIM, 1]); bm3 = par("bm3", [1, 1])
    out = nc.declare_dram_parameter("out", [1, NLOC], FP, isOutput=True)

    xl_loc = {l: nc.dram_tensor(f"xl_loc{l}", [NLOC, HID], BF) for l in (1, 2, 3)}
    xl_full = {l: nc.dram_tensor(f"xl_full{l}", [N, HID], BF, addr_space="Shared")
               for l in (1, 2, 3)}

    with tile.TileContext(nc) as tc:
        with (
            tc.tile_pool(name="const", bufs=1) as cp,
            tc.tile_pool(name="big", bufs=1) as bigp,
            tc.tile_pool(name="wk", bufs=1) as wk,
            tc.tile_pool(name="ps_mm", bufs=2, space="PSUM") as ps_mm,
            tc.tile_pool(name="ps_w", bufs=3, space="PSUM") as ps_w,
            tc.tile_pool(name="ps_o", bufs=2, space="PSUM") as ps_o_pool,
            tc.tile_pool(name="ps_tp", bufs=1, space="PSUM") as ps_tp,
        ):
            def load_const(pname, ap, shape, dtype=BF):
                t = cp.tile(list(shape), dtype, name=pname + "_sb")
                nc.sync.dma_start(out=t[:], in_=ap[:])
                return t

            def load_const_2k(pname, ap, cols):
                t = cp.tile([P, 2 * cols], BF, name=pname + "_sb")
                nc.sync.dma_start(out=t[:, :cols], in_=ap[:P, :])
                nc.sync.dma_start(out=t[:, cols:], in_=ap[P:, :])
                return t

            ident_f = cp.tile([P, P], FP, name="ident_f")
            make_identity(nc, ident_f[:])
            ident_b = cp.tile([P, P], BF, name="ident_b")
            nc.vector.tensor_copy(out=ident_b[:], in_=ident_f[:])
            ones_row = cp.tile([1, 512], BF, name="ones_row")
            nc.vector.memset(ones_row[:], 1.0)
            ones_col = cp.tile([P, 1], BF, name="ones_col")
            nc.vector.memset(ones_col[:], 1.0)

            xT_sb = load_const("xT", xT, [F_IN, NLOC])
            idx_sb = {1: load_const("idx1", idx1, [P, tot_idx_cols], I16),
                      2: load_const("idx23", idx23, [P, tot_idx_cols], I16)}
            idx_sb[3] = idx_sb[2]
            Win_sb = load_const("Win", Win, [F_IN, DIM])
            b_in_sb = load_const("b_in", b_in, [1, DIM])
            Wskip_sb = load_const("Wskip", Wskip, [DIM, HID])
            bskip_sb = load_const("bskip", bskip, [1, HID])
            WL_sb = {1: load_const("WL1", WL1, [DIM, HID]),
                     2: load_const_2k("WL2", WL2, HID),
                     3: load_const_2k("WL3", WL3, HID)}
            WR_sb = {1: load_const("WR1", WR1, [DIM, HID]),
                     2: load_const_2k("WR2", WR2, HID),
                     3: load_const_2k("WR3", WR3, HID)}
            sgn_sb = {l: load_const(f"sgn{l}", sgn[l], [P, HID])
                      for l in (1, 2, 3)}
            dinv_sb = {l: load_const(f"dinv{l}", dinv[l], [P, 2], FP)
                       for l in (1, 2, 3)}
            Wm1_sb = load_const_2k("Wm1", Wm1, DIM)
            bm1_sb = load_const("bm1", bm1, [1, DIM])
            Wm2_sb = load_const("Wm2", Wm2, [DIM, DIM])
            bm2_sb = load_const("bm2", bm2, [1, DIM])
            Wm3_sb = load_const("Wm3", Wm3, [DIM, 1])
            bm3_sb = load_const("bm3", bm3, [1, 1])

            # transposed residual stream buffers (feature chunk k on partitions)
            sT = [bigp.tile([P, NLOC], BF, name=f"sT{k}") for k in range(2)]
            aT = [bigp.tile([P, NLOC], BF, name=f"aT{k}") for k in range(2)]
            bT = [bigp.tile([P, NLOC], BF, name=f"bT{k}") for k in range(2)]
            XRb = {0: bigp.tile([P, NBLK * HID], BF, name="XRb0"),
                   1: bigp.tile([P, NBLK * HID], BF, name="XRb1")}
            hT = bigp.tile([DIM, NLOC], BF, name="hT")
            m1T = bigp.tile([DIM, NLOC], BF, name="m1T")
            m2T = bigp.tile([DIM, NLOC], BF, name="m2T")
            y_sb = bigp.tile([1, NLOC], FP, name="y_sb")

            idx_off = [0]
            nt_off = [0]
            for s in range(NBLK):
                idx_off.append(idx_off[-1] + EBLK[s] // 16)
                nt_off.append(nt_off[-1] + NT_slots[s])

            def ag(l, s):
                if l == 1:                 # slot-major table, one collective
                    if s == NBLK - 1:
                        nc.gpsimd.collective_compute(
                            "AllGather", ALU.bypass,
                            replica_groups=[list(range(NCORES))],
                            ins=[xl_loc[1][:]], outs=[xl_full[1][:]])
                    return
                if s + 1 not in AG_CH:
                    return
                q = AG_CH.index(s + 1) - 1
                lo, hi = AG_CH[q] * P, AG_CH[q + 1] * P
                nc.gpsimd.collective_compute(
                    "AllGather", ALU.bypass,
                    replica_groups=[list(range(NCORES))],
                    ins=[xl_loc[l][lo:hi, :]],
                    outs=[xl_full[l][lo * NCORES:hi * NCORES, :]])

            def xlxr_slot(l, s, src0, src1):
                """xl/xr for layer l, slot s, from actT chunks src0/src1
                (or hT when l == 1); stores xl row-block, fills XRb."""
                nsl = slice(s * P, (s + 1) * P)
                pxl = ps_mm.tile([P, HID], FP, space="PSUM", name="pxl", tag="pmm")
                pxr = ps_mm.tile([P, HID], FP, space="PSUM", name="pxr", tag="pmm")
                if l == 1:
                    nc.tensor.matmul(out=pxl[:], lhsT=hT[:DIM, nsl], rhs=WL_sb[1][:],
                                     start=True, stop=True)
                    nc.tensor.matmul(out=pxr[:], lhsT=hT[:DIM, nsl], rhs=WR_sb[1][:],
                                     start=True, stop=True)
                else:
                    srcs = (src0, src1)
                    for k in range(2):
                        nc.tensor.matmul(out=pxl[:], lhsT=srcs[k][:, nsl],
                                         rhs=WL_sb[l][:, k * HID:(k + 1) * HID],
                                         start=(k == 0), stop=(k == 1))
                    for k in range(2):
                        nc.tensor.matmul(out=pxr[:], lhsT=srcs[k][:, nsl],
                                         rhs=WR_sb[l][:, k * HID:(k + 1) * HID],
                                         start=(k == 0), stop=(k == 1))
                xst = wk.tile([P, HID], BF, name="xst", tag="xst", bufs=4)
                nc.scalar.activation(out=xst[:], in_=pxl[:], func=AF.Copy)
                nc.sync.dma_start(out=xl_loc[l][nsl, :], in_=xst[:])
                nc.scalar.activation(out=XRb[l % 2][:, s * HID:(s + 1) * HID],
                                     in_=pxr[:], func=AF.Copy)

            # ---------------- phase A ----------------
            for j in range(NLOC // 512):
                sl = slice(j * 512, (j + 1) * 512)
                pmm = ps_mm.tile([P, 512], FP, space="PSUM", name="pmm", tag="pmm")
                nc.tensor.matmul(out=pmm[:DIM, :], lhsT=Win_sb[:], rhs=xT_sb[:, sl],
                                 start=True, stop=False)
                nc.tensor.matmul(out=pmm[:DIM, :], lhsT=b_in_sb[:], rhs=ones_row[:],
                                 start=False, stop=True)
                nc.scalar.activation(out=hT[:DIM, sl], in_=pmm[:DIM, :], func=AF.Relu)

            for s in range(NBLK):
                xlxr_slot(1, s, None, None)
                ag(1, s)

            # skip projection overlaps the layer-1 AllGather
            for k in range(2):
                ksl = slice(k * P, (k + 1) * P)
                for j in range(NLOC // 512):
                    sl = slice(j * 512, (j + 1) * 512)
                    psk = ps_mm.tile([P, 512], FP, space="PSUM", name="psk", tag="pmm")
                    nc.tensor.matmul(out=psk[:], lhsT=Wskip_sb[:, ksl],
                                     rhs=hT[:DIM, sl], start=True, stop=False)
                    nc.tensor.matmul(out=psk[:], lhsT=bskip_sb[:, ksl],
                                     rhs=ones_row[:], start=False, stop=True)
                    nc.scalar.activation(out=sT[k][:, sl], in_=psk[:], func=AF.Copy)

            # ---------------- edge stage ----------------
            def edge_layer(l, H, prev0, prev1, next0, next1):
                for s in range(NBLK):
                    NT = NT_slots[s]
                    m_sb = wk.tile([P, NTMAX * P], BF, name="m_sb",
                                   tag="m_sb", bufs=3)
                    nc.sync.dma_start(
                        out=m_sb[:, :NT * P],
                        in_=m_dram[:, nt_off[s] * P:nt_off[s + 1] * P])
                    mT_sb = wk.tile([P, NTMAX * P], BF, name="mT_sb",
                                    tag="mT_sb", bufs=3)
                    nc.sync.dma_start(
                        out=mT_sb[:, :NT * P],
                        in_=mT_dram[:, nt_off[s] * P:nt_off[s + 1] * P])
                    xl_all = wk.tile([P, NTMAX * HID], BF, name="xl_all",
                                     tag="xl_all", bufs=3)
                    for t0 in range(0, NT, 8):
                        ntc = min(8, NT - t0)
                        nc.gpsimd.dma_gather(
                            xl_all[:, t0 * HID:(t0 + ntc) * HID]
                                .rearrange("p (t c) -> p t c", c=HID),
                            xl_full[l][:],
                            idx_sb[l][:, idx_off[s] + t0 * 8:
                                      idx_off[s] + (t0 + ntc) * 8],
                            ntc * P, ntc * P, HID,
                            queue_num=1 + ((s + t0 // 8) % 3))
                    lg = wk.tile([P, NTMAX * 4], FP, name="lg", tag="lg", bufs=2)
                    lj = wk.tile([P, NTMAX], FP, name="lj", tag="lj", bufs=2)
                    for t in range(NT):
                        psw = ps_w.tile([P, HID], FP, space="PSUM",
                                        name="psw", tag="psw")
                        nc.tensor.matmul(out=psw[:],
                                         lhsT=mT_sb[:, t * P:(t + 1) * P],
                                         rhs=XRb[l % 2][:, s * HID:(s + 1) * HID],
                                         start=True, stop=False)
                        nc.tensor.matmul(out=psw[:], lhsT=ident_b[:],
                                         rhs=xl_all[:, t * HID:(t + 1) * HID],
                                         start=False, stop=True)
                        lk = wk.tile([P, HID], BF, name="lk", tag="lk", bufs=4)
                        slk = wk.tile([P, HID], BF, name="slk", tag="slk", bufs=4)
                        if H == 4:
                            nc.scalar.activation(out=lk[:], in_=psw[:],
                                                 func=AF.Prelu, alpha=NEG)
                            nc.vector.scalar_tensor_tensor(
                                out=slk[:], in0=lk[:], scalar=0.0,
                                in1=sgn_sb[l][:],
                                op0=ALU.bypass, op1=ALU.mult,
                                accum_out=lj[:, t:t + 1])
                            nc.vector.tensor_reduce(
                                out=lg[:, 4 * t:4 * t + 4],
                                in_=slk[:].rearrange("p (h d) -> p h d", h=4),
                                axis=AX.X, op=ALU.add)
                        else:
                            nc.scalar.activation(out=lk[:], in_=psw[:],
                                                 func=AF.Prelu, alpha=NEG)
                            nc.vector.scalar_tensor_tensor(
                                out=slk[:, 0:1].to_broadcast([P, HID]),
                                in0=lk[:], scalar=0.0,
                                in1=sgn_sb[l][:],
                                op0=ALU.bypass, op1=ALU.mult,
                                accum_out=lg[:, t:t + 1])
                    al = wk.tile([P, NTMAX * 4], FP, name="al", tag="al", bufs=2)
                    nc.scalar.activation(out=al[:, :NT * H], in_=lg[:, :NT * H],
                                         func=AF.Exp)
                    ps_o = ps_o_pool.tile([P, 264], FP, space="PSUM",
                                          name="ps_o", tag="ps_o")
                    if H == 4:
                        v_slot = wk.tile([P, NTMAX * 260], BF, name="v_slot",
                                         tag="v_slot", bufs=2)
                        nc.vector.tensor_copy(
                            out=v_slot[:, :NT * 260]
                                .rearrange("p (t c) -> p t c", c=260)[:, :, HID:260],
                            in_=al[:, :NT * 4].rearrange("p (t h) -> p t h", h=4))
                        for t in range(NT):
                            nc.vector.tensor_tensor(
                                out=v_slot[:, t * 260:t * 260 + HID]
                                    .rearrange("p (h d) -> p h d", h=4),
                                in0=xl_all[:, t * HID:(t + 1) * HID]
                                    .rearrange("p (h d) -> p h d", h=4),
                                in1=al[:, 4 * t:4 * t + 4, None]
                                    .to_broadcast([P, 4, DIM]),
                                op=ALU.mult)
                            nc.tensor.matmul(out=ps_o[:, :260],
                                             lhsT=m_sb[:, t * P:(t + 1) * P],
                                             rhs=v_slot[:, t * 260:(t + 1) * 260],
                                             start=(t == 0), stop=(t == NT - 1))
                    else:
                        v_slot = wk.tile([P, NTMAX * 257], BF, name="v_slot2",
                                         tag="v_slot2", bufs=2)
                        nc.vector.tensor_copy(
                            out=v_slot[:, :NT * 257]
                                .rearrange("p (t c) -> p t c", c=257)[:, :, HID:],
                            in_=al[:, :NT, None])
                        for t in range(NT):
                            nc.vector.tensor_scalar(
                                out=v_slot[:, t * 257:t * 257 + HID],
                                in0=xl_all[:, t * HID:(t + 1) * HID],
                                scalar1=al[:, t:t + 1], scalar2=None,
                                op0=ALU.mult)
                            nc.tensor.matmul(out=ps_o[:, :HID + 1],
                                             lhsT=m_sb[:, t * P:(t + 1) * P],
                                             rhs=v_slot[:, t * 257:(t + 1) * 257],
                                             start=(t == 0), stop=(t == NT - 1))
                    # finalize
                    g_sb = wk.tile([P, HID], BF, name="g_sb", tag="g_sb", bufs=2)
                    if H == 4:
                        rec = wk.tile([P, 4], FP, name="rec", tag="rec", bufs=2)
                        nc.vector.reciprocal(out=rec[:], in_=ps_o[:, HID:HID + 4])
                        gpre = wk.tile([P, HID], BF, name="gpre", tag="gpre",
                                       bufs=2)
                        nc.vector.tensor_tensor(
                            out=gpre[:].rearrange("p (h d) -> p h d", h=4),
                            in0=ps_o[:, :HID].rearrange("p (h d) -> p h d", h=4),
                            in1=rec[:, :, None].to_broadcast([P, 4, DIM]),
                            op=ALU.mult)
                        nc.scalar.activation(out=g_sb[:], in_=gpre[:], func=AF.Relu)
                    else:
                        rec = wk.tile([P, 1], FP, name="rec", tag="rec", bufs=2)
                        nc.vector.reciprocal(out=rec[:], in_=ps_o[:, HID:HID + 1])
                        nc.scalar.activation(out=g_sb[:], in_=ps_o[:, :HID],
                                             func=AF.Relu, scale=rec[:, 0:1])
                    nsl = slice(s * P, (s + 1) * P)
                    nexts = (next0, next1)
                    prevs = (prev0, prev1)
                    tp = ps_tp.tile([P, 2 * P], BF, space="PSUM",
                                    name="tp", tag="tp")
                    for k in range(2):
                        nc.tensor.transpose(out=tp[:, k * P:(k + 1) * P],
                                            in_=g_sb[:, k * P:(k + 1) * P],
                                            identity=ident_b[:])
                    for k in range(2):
                        nc.vector.scalar_tensor_tensor(
                            out=nexts[k][:, nsl], in0=tp[:, k * P:(k + 1) * P],
                            scalar=dinv_sb[l][:, k:k + 1],
                            in1=prevs[k][:, nsl],
                            op0=ALU.mult, op1=ALU.add)
                    if l < 3:
                        xlxr_slot(l + 1, s, next0, next1)
                        ag(l + 1, s)
                    elif s % 4 == 3:
                        phase_c_group(s // 4)

            # MLP head for one 512-node group; interleaved into layer 3's
            # slot loop (group j is ready once slots 4j..4j+3 finalize)
            def phase_c_group(j):
                sl = slice(j * 512, (j + 1) * 512)
                pm1 = ps_mm.tile([P, 512], FP, space="PSUM", name="pm1", tag="pmm")
                for k in range(2):
                    nc.tensor.matmul(out=pm1[:DIM, :],
                                     lhsT=Wm1_sb[:, k * DIM:(k + 1) * DIM],
                                     rhs=sT[k][:, sl], start=(k == 0), stop=False)
                nc.tensor.matmul(out=pm1[:DIM, :], lhsT=bm1_sb[:], rhs=ones_row[:],
                                 start=False, stop=True)
                nc.scalar.activation(out=m1T[:DIM, sl], in_=pm1[:DIM, :], func=AF.Relu)
                pm2 = ps_mm.tile([P, 512], FP, space="PSUM", name="pm2", tag="pmm")
                nc.tensor.matmul(out=pm2[:DIM, :], lhsT=Wm2_sb[:], rhs=m1T[:DIM, sl],
                                 start=True, stop=False)
                nc.tensor.matmul(out=pm2[:DIM, :], lhsT=bm2_sb[:], rhs=ones_row[:],
                                 start=False, stop=True)
                nc.scalar.activation(out=m2T[:DIM, sl], in_=pm2[:DIM, :], func=AF.Relu)
                py = ps_mm.tile([P, 512], FP, space="PSUM", name="py", tag="pmm")
                nc.tensor.matmul(out=py[:1, :], lhsT=Wm3_sb[:], rhs=m2T[:DIM, sl],
                                 start=True, stop=False)
                nc.tensor.matmul(out=py[:1, :], lhsT=bm3_sb[:], rhs=ones_row[:],
                                 start=False, stop=True)
                nc.scalar.activation(out=y_sb[:, sl], in_=py[:1, :], func=AF.Copy)
                nc.sync.dma_start(out=out[:, sl], in_=y_sb[:, sl])

            edge_layer(1, 4, sT[0], sT[1], aT[0], aT[1])
            edge_layer(2, 1, aT[0], aT[1], bT[0], bT[1])
            edge_layer(3, 1, bT[0], bT[1], sT[0], sT[1])

    nc.compile()
    return nc


_BUILD_CACHE = {}


def _get_program(key):
    if key not in _BUILD_CACHE:
        _BUILD_CACHE[key] = _build(key)
    return _BUILD_CACHE[key]


def kernel(**inputs) -> np.ndarray:
    global LAST_RESULTS
    ii = {k: np.asarray(v) for k, v in inputs.items()}
    assert ii["x"].shape == (N, F_IN)
    for l in (1, 2, 3):
        assert not np.any(ii[f"b{l}"]), "GAT bias assumed zero"

    idx1_all, idx23_all, m_host, mT_host, NT_slots, node_order = _prep_edges(
        np.asarray(ii["edge_index"], np.int64))
    w = _prep_weights(ii)

    def bf(a):
        return np.asarray(a, np.float32).astype(BF_NP)

    common = dict(
        Win=bf(ii["Win"]), b_in=bf(ii["b_in"])[None, :],
        Wskip=bf(ii["Wskip"]), bskip=bf(ii["bskip"])[None, :],
        WL1=w["WL1"], WR1=w["WR1"], WL2=w["WL2"], WR2=w["WR2"],
        WL3=w["WL3"], WR3=w["WR3"],
        sgn1=w["sgn1"], sgn2=w["sgn2"], sgn3=w["sgn3"],
        dinv1=w["dinv1"], dinv2=w["dinv2"], dinv3=w["dinv3"],
        Wm1=bf(ii["Wm1"]), bm1=bf(ii["bm1"])[None, :],
        Wm2=bf(ii["Wm2"]), bm2=bf(ii["bm2"])[None, :],
        Wm3=bf(ii["Wm3"]), bm3=bf(ii["bm3"])[None, :],
    )
    x = np.asarray(ii["x"], np.float32)
    in_maps = []
    for c in range(NCORES):
        m = dict(common)
        m["xT"] = np.ascontiguousarray(x[node_order[c]].T).astype(BF_NP)
        m["idx1"] = idx1_all[c]
        m["idx23"] = idx23_all[c]
        m["m_dram"] = m_host[c]
        m["mT_dram"] = mT_host[c]
        in_maps.append(m)

    nc = _get_program(NT_slots)
    res = run_bass_kernel_spmd(nc, in_maps, list(range(NCORES)),
                               trace=bool(os.environ.get("GAT_TRACE")))
    LAST_RESULTS = res
    y = np.empty(N, np.float32)
    for c in range(NCORES):
        y[node_order[c]] = res.results[c]["out"].reshape(-1)
    return y
